# revision 16
# baseline (speedup 1.0000x reference)
"""CMHSA Trainium2 kernel: cross-head-mixed attention with instance norm.

Math (per batch element b, all on one core; B=8 -> 8 cores data-parallel):
  xf [C=256, T=1024]
  q = Wq@xf, k = Wk@xf, v = Wv@xf            (C x T)
  s[h] = sum_g w_head[h,g] * (q_g^T k_g) * sc    -- folded: qt_h = q scaled
         rowwise by w_head[h, c//32]; s^T[h] = k^T @ qt_h  (K=256 contraction)
  E = exp(sc * s^T)  [t, qp] layout, bf16
  AV per head-pair via PE column tiling: head a -> PSUM rows 0:33 (Z | r),
  head b -> rows 64:97; SQ (rs2 = sum_t E^2) likewise rows 0 / 64.
  Per-pair stats pipelined during phase 2 in a DMA-transposed [32, 64]
  layout (16 partitions/head, free=64), so every DVE stat op costs ~64
  cycles instead of 1024:
    rinv = 1/r; m2 = sum_qp rs2*rinv^2 (stt accum + DMA-transpose + reduce)
    var = m2/T^2 - 1/T^2; alpha = gamma*rsqrt(var+eps)
  alpha is folded into the pb broadcast matmul lhsT (alphas row), so the
  per-qp scale s1 = alpha/r never needs a wide DVE op.
  out = alpha*(Zraw/r) + (beta - alpha/T)*vsum  -> projection by Wp with the
  torch raw-view head scramble folded into Wp^T row slicing:
    y_pre[t=128h+m, c=32j+d] = Z_h[q=8m+j, d]
    y^T[o, t] = sum_j Wp^T[32j:32j+32, o] @ Zs_h[:, j::8]  (+ bias fixups)
"""

import math
import os
import warnings

warnings.filterwarnings("ignore")

import numpy as np

import concourse.bass as bass
import concourse.mybir as mybir
import concourse.tile as tile
from concourse import library_config
from concourse.bass_utils import run_bass_kernel_spmd

B, C, T, NH, HD, P = 8, 256, 1024, 8, 32, 128
EPS = 1e-5
SCALE = 1.0 / math.sqrt(HD)
F32 = mybir.dt.float32
F32R = mybir.dt.float32r
BF16 = mybir.dt.bfloat16
AF = mybir.ActivationFunctionType
ALU = mybir.AluOpType
N_CORES = 8


def _r(ap):
    return ap.bitcast(F32R)


def _split_excess_waits(nc, max_waits=1):
    """This walrus build rejects >2 sem-waits on one instruction
    ("Too many sync wait commands" in setupSyncWait). Hoist excess waits
    onto same-engine NoOps inserted right before the offending instruction."""
    for f in nc.m.functions:
        for blk in f.blocks:
            insts = list(blk.instructions)
            out, changed = [], False
            for inst in insts:
                si = inst.sync_info
                waits = list(si.on_wait) if si and si.on_wait else []
                if len(waits) > max_waits:
                    extra, keep = waits[:-max_waits], waits[-max_waits:]
                    for w in extra:
                        nop = mybir.InstNoOp(
                            name=f"I-sw-{nc.next_id()}",
                            ins=[],
                            outs=[],
                            sync_info=mybir.SyncInfo(on_wait=[w], on_update=[]),
                            engine=inst.engine,
                        )
                        nc.register_instruction(nop)
                        out.append(nop)
                    inst.sync_info = mybir.SyncInfo(
                        on_wait=keep, on_update=list(si.on_update or [])
                    )
                    changed = True
                out.append(inst)
            if changed:
                blk.instructions = out


def build_bass(reps=1):
    nc = bass.Bass("TRN2", target_bir_lowering=False, debug=False)

    xf_d = nc.dram_tensor("xf", [C, T], F32R, kind="ExternalInput")
    wqt_d = nc.dram_tensor("wqt", [C, C], F32R, kind="ExternalInput")
    wkt_d = nc.dram_tensor("wkt", [C, C], F32R, kind="ExternalInput")
    wvt_d = nc.dram_tensor("wvt", [C, C], F32R, kind="ExternalInput")
    wst_d = nc.dram_tensor("wst", [C, NH], F32, kind="ExternalInput")
    wptb_d = nc.dram_tensor("wptb", [HD, 8, C], BF16, kind="ExternalInput")
    wpct_d = nc.dram_tensor("wpct", [HD, C], F32R, kind="ExternalInput")
    bp_d = nc.dram_tensor("bp", [C, 1], F32, kind="ExternalInput")
    gmr_d = nc.dram_tensor("gmr", [1, NH], F32, kind="ExternalInput")
    btr_d = nc.dram_tensor("btr", [1, NH], F32, kind="ExternalInput")
    yt_d = nc.dram_tensor("yt", [C, T], F32, kind="ExternalOutput")

    with tile.TileContext(nc) as tc:
        with (
            tc.tile_pool(name="w", bufs=1) as wp,
            tc.tile_pool(name="stream", bufs=2) as sp,
        ):
            # ---- persistent SBUF tensors ----
            xf_sb = [wp.tile([P, T], F32R, name=f"xf{i}", tag=f"xf{i}") for i in range(2)]
            wqt_sb = [wp.tile([P, C], F32R, name=f"wqt{i}", tag=f"wqt{i}") for i in range(2)]
            wkt_sb = [wp.tile([P, C], F32R, name=f"wkt{i}", tag=f"wkt{i}") for i in range(2)]
            wvt_sb = [wp.tile([P, C], F32R, name=f"wvt{i}", tag=f"wvt{i}") for i in range(2)]
            wst_sb = [wp.tile([P, NH], F32, name=f"wst{i}", tag=f"wst{i}") for i in range(2)]
            wptb_sb = wp.tile([HD, 8, C], BF16, name="wptb", tag="wptb")
            wpct_sb = wp.tile([HD, C], F32R, name="wpct", tag="wpct")
            bp_sb = [wp.tile([P, 1], F32, name=f"bp{i}", tag=f"bp{i}") for i in range(2)]
            gmr_sb = wp.tile([1, NH], F32, name="gmr", tag="gmr")
            btr_sb = wp.tile([1, NH], F32, name="btr", tag="btr")
            q_sb = [wp.tile([P, T], BF16, name=f"q{i}", tag=f"q{i}") for i in range(2)]
            k_sb = [wp.tile([P, T], BF16, name=f"k{i}", tag=f"k{i}") for i in range(2)]
            qt_all = [
                wp.tile([P, NH, T], BF16, name=f"qt{i}", tag=f"qt{i}") for i in range(2)
            ]
            # v^T tiles: per t-chunk tm, 8 heads x (32 cols + ones col)
            vt_sb = wp.tile([P, 8, NH * 33], BF16, name="vt", tag="vt")
            ones_col = wp.tile([P, 1], BF16, name="ones", tag="ones")
            # Z rows 0..31, r row 32 per head block of 1024 qp
            zr_sb = wp.tile([33, NH * T], BF16, name="zr", tag="zr")
            s1all = wp.tile([1, NH * T], F32R, name="s1all", tag="s1all")
            r_row = [
                wp.tile([1, T], BF16, name=f"rrow{i}", tag=f"rrow{i}") for i in range(NH)
            ]
            rs2row = [
                wp.tile([1, T], F32, name=f"rs2row{i}", tag=f"rs2row{i}")
                for i in range(NH)
            ]
            ssall = wp.tile([1, 4 * 32], F32, name="ssall", tag="ssall")
            m2row = wp.tile([1, NH], F32, name="m2row", tag="m2row")
            vprow = wp.tile([1, NH], F32, name="vprow", tag="vprow")
            sdrow = wp.tile([1, NH], F32, name="sdrow", tag="sdrow")
            rdrow = wp.tile([1, NH], F32, name="rdrow", tag="rdrow")
            alpharow = wp.tile([1, NH], F32, name="alpharow", tag="alpharow")
            biasrow = wp.tile([1, NH], F32R, name="biasrow", tag="biasrow")
            alphas = wp.tile([1, NH * HD], F32R, name="alphas", tag="alphas")
            vs_row = wp.tile([1, C], F32R, name="vsrow", tag="vsrow")
            vs_dh = wp.tile([HD, NH], F32R, name="vsdh", tag="vsdh")
            u_sb = [wp.tile([P, NH], F32, name=f"u{i}", tag=f"u{i}") for i in range(2)]
            y_sb = [wp.tile([P, T], F32, name=f"y{i}", tag=f"y{i}") for i in range(2)]
            ones_colr = wp.tile([P, 1], F32R, name="ones_colr", tag="ones_colr")
            onesr128 = wp.tile([1, P], F32R, name="onesr128", tag="onesr128")
            onesf128 = wp.tile([1, P], F32, name="onesf128", tag="onesf128")

            def _one_rep():
                # ---- phase 0: loads ----
                for i in range(2):
                    nc.sync.dma_start(wqt_sb[i][:], wqt_d[i * P : (i + 1) * P, :])
                    nc.sync.dma_start(xf_sb[i][:], xf_d[i * P : (i + 1) * P, :])
                for i in range(2):
                    nc.sync.dma_start(wkt_sb[i][:], wkt_d[i * P : (i + 1) * P, :])
                    nc.gpsimd.dma_start(wvt_sb[i][:], wvt_d[i * P : (i + 1) * P, :])
                    nc.gpsimd.dma_start(wst_sb[i][:], wst_d[i * P : (i + 1) * P, :])
                    nc.gpsimd.dma_start(bp_sb[i][:], bp_d[i * P : (i + 1) * P, :])
                nc.gpsimd.dma_start(wptb_sb[:], wptb_d[:])
                nc.gpsimd.dma_start(wpct_sb[:], wpct_d[:])
                nc.gpsimd.dma_start(gmr_sb[:], gmr_d[:])
                nc.gpsimd.dma_start(btr_sb[:], btr_d[:])
                nc.vector.memset(ones_col[:], 1.0)
                nc.vector.memset(onesf128[:], 1.0)
                nc.vector.tensor_copy(ones_colr[:], ones_col[:])
                nc.vector.tensor_copy(onesr128[:], onesf128[:])
                # ones columns inside vt (lhsT column 32 of each head block)
                for tm8 in range(8):
                    vt3 = vt_sb[:, tm8, :].rearrange("p (h e) -> p h e", e=33)
                    nc.vector.tensor_copy(
                        vt3[:, :, 32:33], ones_col[:, 0:1].broadcast_to((P, 8, 1))
                    )

                # ---- phase 1: projections ----
                with tc.tile_pool(name="psA", bufs=2, space=bass.MemorySpace.PSUM) as psA:
                    for wt, dst in ((wqt_sb, q_sb), (wkt_sb, k_sb)):
                        for co in range(2):
                            for tn in range(2):
                                pq = psA.tile([P, 512], F32, name="qk", tag="qk")
                                for kc in range(2):
                                    nc.tensor.matmul(
                                        pq[:],
                                        wt[kc][:, co * P : (co + 1) * P],
                                        xf_sb[kc][:, tn * 512 : (tn + 1) * 512],
                                        start=(kc == 0),
                                        stop=(kc == 1),
                                    )
                                nc.scalar.activation(
                                    dst[co][:, tn * 512 : (tn + 1) * 512], pq[:], AF.Copy
                                )
                    # qt for all heads (bf16, per-partition wst scale)
                    for kc in range(2):
                        for h in range(NH):
                            nc.vector.tensor_scalar_mul(
                                qt_all[kc][:, h, :], q_sb[kc][:], wst_sb[kc][:, h : h + 1]
                            )
                    # v^T = xf^T @ Wv^T, written per t-chunk with head-stride 33
                    pvs = psA.tile([1, C], F32, name="vs", tag="vs")
                    for tm in range(8):
                        pv = psA.tile([P, C], F32, name="vt", tag="vt")
                        for kc in range(2):
                            nc.tensor.matmul(
                                pv[:],
                                xf_sb[kc][:, tm * P : (tm + 1) * P],
                                wvt_sb[kc][:],
                                start=(kc == 0),
                                stop=(kc == 1),
                            )
                        src = pv[:].rearrange("p (h d) -> p h d", h=NH)
                        dst3 = vt_sb[:, tm, :].rearrange("p (h e) -> p h e", e=33)
                        nc.scalar.activation(dst3[:, :, 0:32], src[:], AF.Copy)
                        nc.tensor.matmul(
                            pvs[:],
                            ones_col[:],
                            dst3[:, :, 0:32].rearrange("p h d -> p d h"),
                            start=(tm == 0),
                            stop=(tm == 7),
                        )
                    nc.scalar.activation(vs_row[:], pvs[:], AF.Copy)
                    nc.gpsimd.dma_start(vs_dh[:], vs_row[:])

                # ---- phase 2: streaming attention ----
                def head_stats(h):
                    # rinv row for pb matmul rhs (f32r), plus m2 partial
                    with nc.allow_low_precision(reason="rinv f32r for pb matmul"):
                        nc.vector.reciprocal(
                            s1all[0:1, h * T : (h + 1) * T], r_row[h][:]
                        )
                    w_row = sp.tile([1, T], F32, name="wrow", tag="wrow", bufs=2)
                    j_row = sp.tile([1, T], F32, name="jrow", tag="jrow", bufs=2)
                    if h >= 6:
                        nc.vector.tensor_mul(
                            w_row[:],
                            rs2row[h][:],
                            s1all[0:1, h * T : (h + 1) * T],
                        )
                        nc.vector.scalar_tensor_tensor(
                            j_row[:], w_row[:], 1.0,
                            s1all[0:1, h * T : (h + 1) * T],
                            op0=ALU.mult, op1=ALU.mult,
                            accum_out=m2row[0:1, h : h + 1],
                        )
                    else:
                        nc.gpsimd.tensor_mul(
                            w_row[:],
                            rs2row[h][:],
                            s1all[0:1, h * T : (h + 1) * T],
                        )
                        nc.gpsimd.tensor_mul(
                            j_row[:], w_row[:], s1all[0:1, h * T : (h + 1) * T]
                        )
                        nc.gpsimd.tensor_reduce(
                            m2row[0:1, h : h + 1], j_row[:],
                            axis=mybir.AxisListType.XYZWC, op=ALU.add,
                        )

                def pair_alpha(p):
                    nc.vector.tensor_scalar(
                        vprow[0:1, 2 * p : 2 * p + 2],
                        m2row[0:1, 2 * p : 2 * p + 2],
                        1.0 / (T * T), EPS - 1.0 / (T * T),
                        op0=ALU.mult, op1=ALU.add,
                    )
                    nc.scalar.activation(
                        sdrow[0:1, 2 * p : 2 * p + 2],
                        vprow[0:1, 2 * p : 2 * p + 2],
                        AF.Sqrt,
                    )
                    nc.vector.reciprocal(
                        rdrow[0:1, 2 * p : 2 * p + 2],
                        sdrow[0:1, 2 * p : 2 * p + 2],
                    )
                    nc.vector.tensor_mul(
                        alpharow[0:1, 2 * p : 2 * p + 2],
                        rdrow[0:1, 2 * p : 2 * p + 2],
                        gmr_sb[0:1, 2 * p : 2 * p + 2],
                    )
                    with nc.allow_low_precision(reason="bias row f32r for pbb matmul"):
                        nc.vector.scalar_tensor_tensor(
                            biasrow[0:1, 2 * p : 2 * p + 2],
                            alpharow[0:1, 2 * p : 2 * p + 2],
                            -1.0 / T,
                            btr_sb[0:1, 2 * p : 2 * p + 2],
                            op0=ALU.mult, op1=ALU.add,
                        )
                    nc.vector.tensor_copy(
                        alphas[0:1, 2 * p * HD : (2 * p + 2) * HD].rearrange(
                            "o (h d) -> o h d", h=2
                        ),
                        alpharow[0:1, 2 * p : 2 * p + 2][:, :, None].broadcast_to(
                            (1, 2, HD)
                        ),
                    )

                with (
                    tc.tile_pool(name="psS", bufs=2, space=bass.MemorySpace.PSUM) as psS,
                    tc.tile_pool(name="psAV", bufs=2, space=bass.MemorySpace.PSUM) as psAV,
                ):
                    for h in range(NH):
                        pav = psAV.tile([65, T], F32, name="pav", tag="pav")
                        for tm in range(8):
                            ps = psS.tile([P, T], F32, name="s", tag="s")
                            for kc in range(2):
                                for qh in range(2):
                                    nc.tensor.matmul(
                                        ps[:, qh * 512 : (qh + 1) * 512],
                                        k_sb[kc][:, tm * P : (tm + 1) * P],
                                        qt_all[kc][:, h, qh * 512 : (qh + 1) * 512],
                                        start=(kc == 0),
                                        stop=(kc == 1),
                                    )
                            et = sp.tile([P, T], BF16, name="E", tag="E", bufs=3)
                            nc.scalar.activation(et[:], ps[:], AF.Exp, scale=SCALE)
                            sq = sp.tile([P, T], BF16, name="SQ", tag="SQ", bufs=3)
                            nc.vector.tensor_mul(sq[:], et[:], et[:])
                            for qh in range(2):
                                sl = slice(qh * 512, (qh + 1) * 512)
                                nc.tensor.matmul(
                                    pav[0:33, sl],
                                    vt_sb[:, tm, 33 * h : 33 * h + 33],
                                    et[:, sl],
                                    start=(tm == 0),
                                    stop=(tm == 7),
                                    skip_group_check=True,
                                )
                                nc.tensor.matmul(
                                    pav[64:65, sl],
                                    ones_col[:],
                                    sq[:, sl],
                                    start=(tm == 0),
                                    stop=(tm == 7),
                                    skip_group_check=True,
                                )
                            if h >= 3 and h % 2 == 1 and tm == 4:
                                pair_alpha(h // 2 - 1)
                        nc.scalar.activation(
                            zr_sb[0:33, h * T : (h + 1) * T], pav[0:33, :], AF.Copy
                        )
                        nc.vector.tensor_copy(r_row[h][:], pav[32:33, :])
                        nc.vector.tensor_copy(rs2row[h][:], pav[64:65, :])
                        head_stats(h)
                    pair_alpha(3)

                # ---- tail: u fixups, Z scaling, projection ----
                with (
                    tc.tile_pool(name="psB", bufs=2, space=bass.MemorySpace.PSUM) as psB,
                    tc.tile_pool(name="psY", bufs=2, space=bass.MemorySpace.PSUM) as psY,
                ):
                    # u (Wp colsum @ vsum part; independent of stats)
                    us = []
                    for oc in range(2):
                        pu = psB.tile([P, NH], F32, name="pu", tag="pb")
                        nc.tensor.matmul(
                            pu[:],
                            wpct_sb[:, oc * P : (oc + 1) * P],
                            vs_dh[:],
                            start=True,
                            stop=True,
                        )
                        nc.scalar.activation(u_sb[oc][:], pu[:], AF.Copy)
                    zrr = zr_sb[0:32, :].rearrange("p (h m j) -> p h m j", h=NH, j=8)
                    py = [
                        psY.tile([P, T], F32, name=f"py{oc}", tag="py") for oc in range(2)
                    ]
                    pbs_pool = []
                    for h in range(NH):
                        pbp = psB.tile([32, T], F32, name="pb", tag="pb")
                        for qh in range(2):
                            nc.tensor.matmul(
                                pbp[:, qh * 512 : (qh + 1) * 512],
                                alphas[0:1, h * HD : (h + 1) * HD],
                                s1all[0:1, h * T + qh * 512 : h * T + (qh + 1) * 512],
                                start=True,
                                stop=True,
                            )
                        pbs = sp.tile([32, T], BF16, name="pbs", tag="pbs", bufs=2)
                        nc.scalar.activation(pbs[:], pbp[:], AF.Copy)
                        nc.vector.tensor_mul(
                            zr_sb[0:32, h * T : (h + 1) * T],
                            zr_sb[0:32, h * T : (h + 1) * T],
                            pbs[:],
                        )
                        if h == 3 or h == 7:
                            hf = h // 4
                            for oc in range(2):
                                for j in range(8):
                                    nc.tensor.matmul(
                                        py[oc][:, hf * 512 : (hf + 1) * 512],
                                        wptb_sb[:, j, oc * P : (oc + 1) * P],
                                        zrr[:, 4 * hf : 4 * hf + 4, :, j],
                                        start=(j == 0),
                                        stop=(j == 7),
                                        skip_group_check=True,
                                    )
                    # bias fixup rows -> [128, NH] broadcast via PE
                    pbb = psB.tile([P, NH], F32, name="pbb", tag="pb")
                    nc.tensor.matmul(
                        pbb[:], onesr128[:], biasrow[:], start=True, stop=True
                    )
                    for oc in range(2):
                        nc.vector.tensor_mul(u_sb[oc][:], u_sb[oc][:], pbb[:])
                        nc.vector.tensor_scalar_add(
                            u_sb[oc][:], u_sb[oc][:], bp_sb[oc][:, 0:1]
                        )
                    for hf in range(2):
                        for oc in range(2):
                            yv = y_sb[oc][:].rearrange("p (h m) -> p h m", h=NH)
                            pyv = py[oc][:].rearrange("p (h m) -> p h m", h=NH)
                            bias_b = u_sb[oc][:, 4 * hf : 4 * hf + 4, None].broadcast_to(
                                (P, 4, P)
                            )
                            nc.vector.tensor_add(
                                yv[:, 4 * hf : 4 * hf + 4, :],
                                pyv[:, 4 * hf : 4 * hf + 4, :],
                                bias_b,
                            )
                            eng = nc.sync if oc == 0 else nc.gpsimd
                            eng.dma_start(
                                yt_d[
                                    oc * P : (oc + 1) * P,
                                    hf * 512 : (hf + 1) * 512,
                                ],
                                y_sb[oc][:, hf * 512 : (hf + 1) * 512],
                            )

            for _rep in range(reps):
                _one_rep()

    _split_excess_waits(nc)
    return nc


def _host_inputs(x, Wq, Wk, Wv, w_head, gamma, beta, Wp, bp):
    import ml_dtypes

    f = np.float32
    common = {
        "wqt": np.ascontiguousarray(np.asarray(Wq, f).T),
        "wkt": np.ascontiguousarray(np.asarray(Wk, f).T),
        "wvt": np.ascontiguousarray(np.asarray(Wv, f).T),
        "wst": np.ascontiguousarray(np.repeat(np.asarray(w_head, f), HD, axis=1).T),
        "wptb": np.ascontiguousarray(
            np.asarray(Wp, f).T.reshape(8, HD, C).transpose(1, 0, 2)
        ).astype(ml_dtypes.bfloat16),
        "wpct": np.ascontiguousarray(
            np.asarray(Wp, f).T.reshape(8, HD, C).sum(0)
        ),
        "bp": np.ascontiguousarray(np.asarray(bp, f).reshape(C, 1)),
        "gmr": np.ascontiguousarray(np.asarray(gamma, f).reshape(1, NH)),
        "btr": np.ascontiguousarray(np.asarray(beta, f).reshape(1, NH)),
    }
    xs = np.asarray(x, f).reshape(B, C, T)
    return [
        {"xf": np.ascontiguousarray(xs[b]), **common} for b in range(B)
    ]


_NC_CACHE = {}


def _get_nc(reps=1):
    if reps not in _NC_CACHE:
        _NC_CACHE[reps] = build_bass(reps=reps)
    return _NC_CACHE[reps]


def run(inputs, trace=False):
    nc = _get_nc()
    in_maps = _host_inputs(**inputs)
    res = run_bass_kernel_spmd(
        nc, in_maps, core_ids=list(range(N_CORES)), trace=trace
    )
    y = np.stack([res.results[b]["yt"] for b in range(B)], axis=0)
    return y.reshape(B, C, 32, 32).astype(np.float32), res


def _build_sharded(reps=1):
    """Replicate bass2jax.run_bass_via_pjrt but return a reusable callable
    (no donation) so device execution can be timed over many iterations."""
    import jax
    from jax.sharding import Mesh, PartitionSpec
    from jax.experimental.shard_map import shard_map
    from concourse import bass2jax

    nc = _get_nc(reps)
    bass2jax.install_neuronx_cc_hook()
    part_name = nc.partition_id_tensor.name if nc.partition_id_tensor else None
    in_names, out_names, out_avals = [], [], []
    for alloc in nc.m.functions[0].allocations:
        if not isinstance(alloc, mybir.MemoryLocationSet):
            continue
        name = alloc.memorylocations[0].name
        if alloc.kind == "ExternalInput":
            if name == part_name:
                continue
            in_names.append(name)
        elif alloc.kind == "ExternalOutput":
            out_names.append(name)
            out_avals.append(
                jax.core.ShapedArray(
                    tuple(alloc.tensor_shape), mybir.dt.np(alloc.dtype)
                )
            )
    n_params = len(in_names)
    all_in = in_names + out_names
    if part_name is not None:
        all_in = all_in + [part_name]

    def _body(*args):
        operands = list(args)
        if part_name is not None:
            operands.append(bass2jax.partition_id_tensor())
        outs = bass2jax._bass_exec_p.bind(
            *operands,
            out_avals=tuple(out_avals),
            in_names=tuple(all_in),
            out_names=tuple(out_names),
            lowering_input_output_aliases=(),
            sim_require_finite=True,
            sim_require_nnan=True,
            nc=nc,
        )
        return tuple(outs)

    devices = jax.devices()[:N_CORES]
    mesh = Mesh(np.asarray(devices), ("core",))
    nouts = len(out_names)
    sharded = jax.jit(
        shard_map(
            _body,
            mesh=mesh,
            in_specs=(PartitionSpec("core"),) * (n_params + nouts),
            out_specs=(PartitionSpec("core"),) * nouts,
            check_rep=False,
        ),
        keep_unused=True,
    )
    return sharded, mesh, in_names, out_names, out_avals


def timed_run(inputs, iters=20, reps=1):
    import time
    import jax
    from jax.sharding import NamedSharding, PartitionSpec

    sharded, mesh, in_names, out_names, out_avals = _build_sharded(reps)
    in_maps = _host_inputs(**inputs)
    sh = NamedSharding(mesh, PartitionSpec("core"))
    dev_in = [
        jax.device_put(
            np.concatenate([in_maps[c][n] for c in range(N_CORES)], axis=0), sh
        )
        for n in in_names
    ]
    dev_zero = [
        jax.device_put(
            np.zeros((N_CORES * a.shape[0], *a.shape[1:]), a.dtype), sh
        )
        for a in out_avals
    ]
    out = sharded(*dev_in, *dev_zero)
    jax.block_until_ready(out)
    # blocking per-call (includes full dispatch round trip)
    times = []
    for _ in range(max(3, iters // 4)):
        t0 = time.perf_counter()
        out = sharded(*dev_in, *dev_zero)
        jax.block_until_ready(out)
        times.append(time.perf_counter() - t0)
    # pipelined: submit all, block once -> amortizes host/axon dispatch
    t0 = time.perf_counter()
    outs = [sharded(*dev_in, *dev_zero) for _ in range(iters)]
    jax.block_until_ready(outs)
    pipelined = (time.perf_counter() - t0) / iters
    times.append(pipelined)
    print(f"pipelined per-call: {pipelined * 1e9:.0f} ns")
    y = np.asarray(outs[-1][out_names.index("yt")]).reshape(N_CORES, C, T)
    return y.reshape(B, C, 32, 32).astype(np.float32), times


def kernel(**inputs):
    y, _ = run(inputs, trace=False)
    return y


def numpy_check():
    """CoreSim single-core check against a numpy reference (core 0 data)."""
    from concourse.bass_interp import CoreSim

    rng = np.random.default_rng(0)
    x = rng.standard_normal((B, C, 32, 32), np.float32)
    Wq = (rng.standard_normal((C, C)) * 0.05).astype(np.float32)
    Wk = (rng.standard_normal((C, C)) * 0.05).astype(np.float32)
    Wv = (rng.standard_normal((C, C)) * 0.05).astype(np.float32)
    w_head = (rng.standard_normal((NH, NH)) * 0.3).astype(np.float32)
    gamma = rng.uniform(0.5, 1.5, NH).astype(np.float32)
    beta = (rng.standard_normal(NH) * 0.1).astype(np.float32)
    Wp = (rng.standard_normal((C, C)) * 0.05).astype(np.float32)
    bp = (rng.standard_normal(C) * 0.05).astype(np.float32)
    inputs = dict(
        x=x, Wq=Wq, Wk=Wk, Wv=Wv, w_head=w_head, gamma=gamma, beta=beta,
        Wp=Wp, bp=bp,
    )

    def ref_np(x, Wq, Wk, Wv, w_head, gamma, beta, Wp, bp):
        Bn, Cn, H, W = x.shape
        Tn = H * W
        hd = Cn // NH
        sc = float(hd) ** -0.5
        xf = x.reshape(Bn, Cn, Tn).astype(np.float64)
        q = np.einsum("oc,bct->bot", Wq, xf).reshape(Bn, NH, hd, Tn)
        k = np.einsum("oc,bct->bot", Wk, xf).reshape(Bn, NH, hd, Tn)
        v = np.einsum("oc,bct->bot", Wv, xf).reshape(Bn, NH, hd, Tn)
        s = np.einsum("bhdq,bhdt->bhqt", q, k) * sc
        s = np.einsum("hg,bgqt->bhqt", w_head.astype(np.float64), s)
        s = s - s.max(axis=-1, keepdims=True)
        e = np.exp(s)
        a = e / e.sum(-1, keepdims=True)
        mean = a.mean(axis=(2, 3), keepdims=True)
        var = a.var(axis=(2, 3), keepdims=True)
        g = gamma.astype(np.float64)[None, :, None, None]
        bt = beta.astype(np.float64)[None, :, None, None]
        a = (a - mean) / np.sqrt(var + EPS) * g + bt
        out = np.einsum("bhqt,bhdt->bhqd", a, v)
        y = out.reshape(Bn, Tn, Cn)
        y = np.einsum("btc,oc->bto", y, Wp.astype(np.float64)) + bp
        return y.transpose(0, 2, 1).reshape(Bn, Cn, H, W)

    expected = ref_np(**inputs)[0]  # core 0

    nc = _get_nc()
    in_maps = _host_inputs(**inputs)
    sim = CoreSim(nc, trace=False)
    for name, arr in in_maps[0].items():
        sim.tensor(name)[:] = arr
    sim.simulate(check_with_hw=False)
    got = np.array(sim.tensor("yt")).reshape(C, 32, 32)
    err = np.abs(got - expected) / (np.abs(expected) + 1e-3)
    scale = np.abs(got - expected).max() / np.abs(expected).max()
    print("max rel err (sim vs numpy f64):", err.max())
    print("mean rel err:", err.mean())
    print("scale-relative absmax:", scale)
    return err.max()


if __name__ == "__main__":
    numpy_check()


# revision 17
# speedup vs baseline: 1.0306x; 1.0306x over previous
"""CMHSA Trainium2 kernel: cross-head-mixed attention with instance norm.

Math (per batch element b, all on one core; B=8 -> 8 cores data-parallel):
  xf [C=256, T=1024]
  q = Wq@xf, k = Wk@xf, v = Wv@xf            (C x T)
  s[h] = sum_g w_head[h,g] * (q_g^T k_g) * sc    -- folded: qt_h = q scaled
         rowwise by w_head[h, c//32]; s^T[h] = k^T @ qt_h  (K=256 contraction)
  E = exp(sc * s^T)  [t, qp] layout, bf16
  AV per head-pair via PE column tiling: head a -> PSUM rows 0:33 (Z | r),
  head b -> rows 64:97; SQ (rs2 = sum_t E^2) likewise rows 0 / 64.
  Per-pair stats pipelined during phase 2 in a DMA-transposed [32, 64]
  layout (16 partitions/head, free=64), so every DVE stat op costs ~64
  cycles instead of 1024:
    rinv = 1/r; m2 = sum_qp rs2*rinv^2 (stt accum + DMA-transpose + reduce)
    var = m2/T^2 - 1/T^2; alpha = gamma*rsqrt(var+eps)
  alpha is folded into the pb broadcast matmul lhsT (alphas row), so the
  per-qp scale s1 = alpha/r never needs a wide DVE op.
  out = alpha*(Zraw/r) + (beta - alpha/T)*vsum  -> projection by Wp with the
  torch raw-view head scramble folded into Wp^T row slicing:
    y_pre[t=128h+m, c=32j+d] = Z_h[q=8m+j, d]
    y^T[o, t] = sum_j Wp^T[32j:32j+32, o] @ Zs_h[:, j::8]  (+ bias fixups)
"""

import math
import os
import warnings

warnings.filterwarnings("ignore")

import numpy as np

import concourse.bass as bass
import concourse.mybir as mybir
import concourse.tile as tile
from concourse import library_config
from concourse.bass_utils import run_bass_kernel_spmd

B, C, T, NH, HD, P = 8, 256, 1024, 8, 32, 128
EPS = 1e-5
SCALE = 1.0 / math.sqrt(HD)
F32 = mybir.dt.float32
F32R = mybir.dt.float32r
BF16 = mybir.dt.bfloat16
AF = mybir.ActivationFunctionType
ALU = mybir.AluOpType
N_CORES = 8


def _r(ap):
    return ap.bitcast(F32R)


def _split_excess_waits(nc, max_waits=1):
    """This walrus build rejects >2 sem-waits on one instruction
    ("Too many sync wait commands" in setupSyncWait). Hoist excess waits
    onto same-engine NoOps inserted right before the offending instruction."""
    for f in nc.m.functions:
        for blk in f.blocks:
            insts = list(blk.instructions)
            out, changed = [], False
            for inst in insts:
                si = inst.sync_info
                waits = list(si.on_wait) if si and si.on_wait else []
                if len(waits) > max_waits:
                    extra, keep = waits[:-max_waits], waits[-max_waits:]
                    for w in extra:
                        nop = mybir.InstNoOp(
                            name=f"I-sw-{nc.next_id()}",
                            ins=[],
                            outs=[],
                            sync_info=mybir.SyncInfo(on_wait=[w], on_update=[]),
                            engine=inst.engine,
                        )
                        nc.register_instruction(nop)
                        out.append(nop)
                    inst.sync_info = mybir.SyncInfo(
                        on_wait=keep, on_update=list(si.on_update or [])
                    )
                    changed = True
                out.append(inst)
            if changed:
                blk.instructions = out


def build_bass(reps=1):
    nc = bass.Bass("TRN2", target_bir_lowering=False, debug=False)

    xf_d = nc.dram_tensor("xf", [C, T], F32R, kind="ExternalInput")
    wqt_d = nc.dram_tensor("wqt", [C, C], F32R, kind="ExternalInput")
    wkt_d = nc.dram_tensor("wkt", [C, C], F32R, kind="ExternalInput")
    wvt_d = nc.dram_tensor("wvt", [C, C], F32R, kind="ExternalInput")
    wst_d = nc.dram_tensor("wst", [C, NH], F32, kind="ExternalInput")
    wptb_d = nc.dram_tensor("wptb", [HD, 8, C], BF16, kind="ExternalInput")
    wpct_d = nc.dram_tensor("wpct", [HD, C], F32R, kind="ExternalInput")
    bp_d = nc.dram_tensor("bp", [C, 1], F32, kind="ExternalInput")
    gmr_d = nc.dram_tensor("gmr", [1, NH], F32, kind="ExternalInput")
    btr_d = nc.dram_tensor("btr", [1, NH], F32, kind="ExternalInput")
    yt_d = nc.dram_tensor("yt", [C, T], F32, kind="ExternalOutput")

    with tile.TileContext(nc) as tc:
        with (
            tc.tile_pool(name="w", bufs=1) as wp,
            tc.tile_pool(name="stream", bufs=2) as sp,
        ):
            # ---- persistent SBUF tensors ----
            xf_sb = [wp.tile([P, T], F32R, name=f"xf{i}", tag=f"xf{i}") for i in range(2)]
            wqt_sb = [wp.tile([P, C], F32R, name=f"wqt{i}", tag=f"wqt{i}") for i in range(2)]
            wkt_sb = [wp.tile([P, C], F32R, name=f"wkt{i}", tag=f"wkt{i}") for i in range(2)]
            wvt_sb = [wp.tile([P, C], F32R, name=f"wvt{i}", tag=f"wvt{i}") for i in range(2)]
            wst_sb = [wp.tile([P, NH], F32, name=f"wst{i}", tag=f"wst{i}") for i in range(2)]
            wptb_sb = wp.tile([HD, 8, C], BF16, name="wptb", tag="wptb")
            wpct_sb = wp.tile([HD, C], F32R, name="wpct", tag="wpct")
            bp_sb = [wp.tile([P, 1], F32, name=f"bp{i}", tag=f"bp{i}") for i in range(2)]
            gmr_sb = wp.tile([1, NH], F32, name="gmr", tag="gmr")
            btr_sb = wp.tile([1, NH], F32, name="btr", tag="btr")
            q_sb = [wp.tile([P, T], BF16, name=f"q{i}", tag=f"q{i}") for i in range(2)]
            k_sb = [wp.tile([P, T], BF16, name=f"k{i}", tag=f"k{i}") for i in range(2)]
            qt_all = [
                wp.tile([P, NH, T], BF16, name=f"qt{i}", tag=f"qt{i}") for i in range(2)
            ]
            # v^T tiles: per t-chunk tm, 8 heads x (32 cols + ones col)
            vt_sb = wp.tile([P, 8, NH * 33], BF16, name="vt", tag="vt")
            ones_col = wp.tile([P, 1], BF16, name="ones", tag="ones")
            # Z rows 0..31, r row 32 per head block of 1024 qp
            zr_sb = wp.tile([33, NH * T], BF16, name="zr", tag="zr")
            s1all = wp.tile([1, NH * T], F32R, name="s1all", tag="s1all")
            r_row = [
                wp.tile([1, T], BF16, name=f"rrow{i}", tag=f"rrow{i}") for i in range(NH)
            ]
            rs2row = [
                wp.tile([1, T], F32, name=f"rs2row{i}", tag=f"rs2row{i}")
                for i in range(NH)
            ]
            ssall = wp.tile([1, 4 * 32], F32, name="ssall", tag="ssall")
            m2row = wp.tile([1, NH], F32, name="m2row", tag="m2row")
            vprow = wp.tile([1, NH], F32, name="vprow", tag="vprow")
            sdrow = wp.tile([1, NH], F32, name="sdrow", tag="sdrow")
            rdrow = wp.tile([1, NH], F32, name="rdrow", tag="rdrow")
            alpharow = wp.tile([1, NH], F32, name="alpharow", tag="alpharow")
            biasrow = wp.tile([1, NH], F32R, name="biasrow", tag="biasrow")
            alphas = wp.tile([1, NH * HD], F32R, name="alphas", tag="alphas")
            vs_row = wp.tile([1, C], F32R, name="vsrow", tag="vsrow")
            vs_dh = wp.tile([HD, NH], F32R, name="vsdh", tag="vsdh")
            u_sb = [wp.tile([P, NH], F32, name=f"u{i}", tag=f"u{i}") for i in range(2)]
            y_sb = [wp.tile([P, T], F32, name=f"y{i}", tag=f"y{i}") for i in range(2)]
            ones_colr = wp.tile([P, 1], F32R, name="ones_colr", tag="ones_colr")
            onesr128 = wp.tile([1, P], F32R, name="onesr128", tag="onesr128")
            onesf128 = wp.tile([1, P], F32, name="onesf128", tag="onesf128")

            def _one_rep():
                # ---- phase 0: loads ----
                for i in range(2):
                    nc.sync.dma_start(wqt_sb[i][:], wqt_d[i * P : (i + 1) * P, :])
                    nc.sync.dma_start(xf_sb[i][:], xf_d[i * P : (i + 1) * P, :])
                for i in range(2):
                    nc.sync.dma_start(wkt_sb[i][:], wkt_d[i * P : (i + 1) * P, :])
                    nc.gpsimd.dma_start(wvt_sb[i][:], wvt_d[i * P : (i + 1) * P, :])
                    nc.gpsimd.dma_start(wst_sb[i][:], wst_d[i * P : (i + 1) * P, :])
                    nc.gpsimd.dma_start(bp_sb[i][:], bp_d[i * P : (i + 1) * P, :])
                nc.gpsimd.dma_start(wptb_sb[:], wptb_d[:])
                nc.gpsimd.dma_start(wpct_sb[:], wpct_d[:])
                nc.gpsimd.dma_start(gmr_sb[:], gmr_d[:])
                nc.gpsimd.dma_start(btr_sb[:], btr_d[:])
                nc.vector.memset(ones_col[:], 1.0)
                nc.vector.memset(onesf128[:], 1.0)
                nc.vector.tensor_copy(ones_colr[:], ones_col[:])
                nc.vector.tensor_copy(onesr128[:], onesf128[:])
                # ones columns inside vt (lhsT column 32 of each head block)
                for tm8 in range(8):
                    vt3 = vt_sb[:, tm8, :].rearrange("p (h e) -> p h e", e=33)
                    nc.vector.tensor_copy(
                        vt3[:, :, 32:33], ones_col[:, 0:1].broadcast_to((P, 8, 1))
                    )

                # ---- phase 1: projections ----
                with tc.tile_pool(name="psA", bufs=2, space=bass.MemorySpace.PSUM) as psA:
                    for wt, dst in ((wqt_sb, q_sb), (wkt_sb, k_sb)):
                        for co in range(2):
                            for tn in range(2):
                                pq = psA.tile([P, 512], F32, name="qk", tag="qk")
                                for kc in range(2):
                                    nc.tensor.matmul(
                                        pq[:],
                                        wt[kc][:, co * P : (co + 1) * P],
                                        xf_sb[kc][:, tn * 512 : (tn + 1) * 512],
                                        start=(kc == 0),
                                        stop=(kc == 1),
                                    )
                                nc.scalar.activation(
                                    dst[co][:, tn * 512 : (tn + 1) * 512], pq[:], AF.Copy
                                )
                    # qt for all heads (bf16, per-partition wst scale)
                    for kc in range(2):
                        for h in range(NH):
                            nc.vector.tensor_scalar_mul(
                                qt_all[kc][:, h, :], q_sb[kc][:], wst_sb[kc][:, h : h + 1]
                            )
                    # v^T = xf^T @ Wv^T, written per t-chunk with head-stride 33
                    pvs = psA.tile([1, C], F32, name="vs", tag="vs")
                    for tm in range(8):
                        pv = psA.tile([P, C], F32, name="vt", tag="vt")
                        for kc in range(2):
                            nc.tensor.matmul(
                                pv[:],
                                xf_sb[kc][:, tm * P : (tm + 1) * P],
                                wvt_sb[kc][:],
                                start=(kc == 0),
                                stop=(kc == 1),
                            )
                        src = pv[:].rearrange("p (h d) -> p h d", h=NH)
                        dst3 = vt_sb[:, tm, :].rearrange("p (h e) -> p h e", e=33)
                        nc.scalar.activation(dst3[:, :, 0:32], src[:], AF.Copy)
                        nc.tensor.matmul(
                            pvs[:],
                            ones_col[:],
                            dst3[:, :, 0:32].rearrange("p h d -> p d h"),
                            start=(tm == 0),
                            stop=(tm == 7),
                        )
                    nc.scalar.activation(vs_row[:], pvs[:], AF.Copy)
                    nc.gpsimd.dma_start(vs_dh[:], vs_row[:])

                # ---- phase 2: streaming attention ----
                def head_stats(h):
                    # rinv row for pb matmul rhs (f32r), plus m2 partial
                    with nc.allow_low_precision(reason="rinv f32r for pb matmul"):
                        nc.vector.reciprocal(
                            s1all[0:1, h * T : (h + 1) * T], r_row[h][:]
                        )
                    w_row = sp.tile([1, T], F32, name="wrow", tag="wrow", bufs=2)
                    j_row = sp.tile([1, T], F32, name="jrow", tag="jrow", bufs=2)
                    nc.vector.tensor_mul(
                        w_row[:],
                        rs2row[h][:],
                        s1all[0:1, h * T : (h + 1) * T],
                    )
                    nc.vector.scalar_tensor_tensor(
                        j_row[:], w_row[:], 1.0,
                        s1all[0:1, h * T : (h + 1) * T],
                        op0=ALU.mult, op1=ALU.mult,
                        accum_out=m2row[0:1, h : h + 1],
                    )

                def pair_alpha(p):
                    nc.vector.tensor_scalar(
                        vprow[0:1, 2 * p : 2 * p + 2],
                        m2row[0:1, 2 * p : 2 * p + 2],
                        1.0 / (T * T), EPS - 1.0 / (T * T),
                        op0=ALU.mult, op1=ALU.add,
                    )
                    nc.scalar.activation(
                        sdrow[0:1, 2 * p : 2 * p + 2],
                        vprow[0:1, 2 * p : 2 * p + 2],
                        AF.Sqrt,
                    )
                    nc.vector.reciprocal(
                        rdrow[0:1, 2 * p : 2 * p + 2],
                        sdrow[0:1, 2 * p : 2 * p + 2],
                    )
                    nc.vector.tensor_mul(
                        alpharow[0:1, 2 * p : 2 * p + 2],
                        rdrow[0:1, 2 * p : 2 * p + 2],
                        gmr_sb[0:1, 2 * p : 2 * p + 2],
                    )
                    with nc.allow_low_precision(reason="bias row f32r for pbb matmul"):
                        nc.vector.scalar_tensor_tensor(
                            biasrow[0:1, 2 * p : 2 * p + 2],
                            alpharow[0:1, 2 * p : 2 * p + 2],
                            -1.0 / T,
                            btr_sb[0:1, 2 * p : 2 * p + 2],
                            op0=ALU.mult, op1=ALU.add,
                        )
                    nc.vector.tensor_copy(
                        alphas[0:1, 2 * p * HD : (2 * p + 2) * HD].rearrange(
                            "o (h d) -> o h d", h=2
                        ),
                        alpharow[0:1, 2 * p : 2 * p + 2][:, :, None].broadcast_to(
                            (1, 2, HD)
                        ),
                    )

                with (
                    tc.tile_pool(name="psS", bufs=2, space=bass.MemorySpace.PSUM) as psS,
                    tc.tile_pool(name="psAV", bufs=2, space=bass.MemorySpace.PSUM) as psAV,
                ):
                    for h in range(NH):
                        pav = psAV.tile([65, T], F32, name="pav", tag="pav")
                        for tm in range(8):
                            ps = psS.tile([P, T], F32, name="s", tag="s")
                            for kc in range(2):
                                for qh in range(2):
                                    nc.tensor.matmul(
                                        ps[:, qh * 512 : (qh + 1) * 512],
                                        k_sb[kc][:, tm * P : (tm + 1) * P],
                                        qt_all[kc][:, h, qh * 512 : (qh + 1) * 512],
                                        start=(kc == 0),
                                        stop=(kc == 1),
                                    )
                            et = sp.tile([P, T], BF16, name="E", tag="E", bufs=3)
                            nc.scalar.activation(et[:], ps[:], AF.Exp, scale=SCALE)
                            sq = sp.tile([P, T], BF16, name="SQ", tag="SQ", bufs=3)
                            nc.vector.tensor_mul(sq[:], et[:], et[:])
                            for qh in range(2):
                                sl = slice(qh * 512, (qh + 1) * 512)
                                nc.tensor.matmul(
                                    pav[0:33, sl],
                                    vt_sb[:, tm, 33 * h : 33 * h + 33],
                                    et[:, sl],
                                    start=(tm == 0),
                                    stop=(tm == 7),
                                    skip_group_check=True,
                                )
                                nc.tensor.matmul(
                                    pav[64:65, sl],
                                    ones_col[:],
                                    sq[:, sl],
                                    start=(tm == 0),
                                    stop=(tm == 7),
                                    skip_group_check=True,
                                )
                            if h >= 3 and h % 2 == 1 and tm == 4:
                                pair_alpha(h // 2 - 1)
                        nc.scalar.activation(
                            zr_sb[0:33, h * T : (h + 1) * T], pav[0:33, :], AF.Copy
                        )
                        nc.vector.tensor_copy(r_row[h][:], pav[32:33, :])
                        nc.vector.tensor_copy(rs2row[h][:], pav[64:65, :])
                        head_stats(h)
                    pair_alpha(3)

                # ---- tail: u fixups, Z scaling, projection ----
                with (
                    tc.tile_pool(name="psB", bufs=2, space=bass.MemorySpace.PSUM) as psB,
                    tc.tile_pool(name="psY", bufs=2, space=bass.MemorySpace.PSUM) as psY,
                ):
                    # u (Wp colsum @ vsum part; independent of stats)
                    us = []
                    for oc in range(2):
                        pu = psB.tile([P, NH], F32, name="pu", tag="pb")
                        nc.tensor.matmul(
                            pu[:],
                            wpct_sb[:, oc * P : (oc + 1) * P],
                            vs_dh[:],
                            start=True,
                            stop=True,
                        )
                        nc.scalar.activation(u_sb[oc][:], pu[:], AF.Copy)
                    zrr = zr_sb[0:32, :].rearrange("p (h m j) -> p h m j", h=NH, j=8)
                    py = [
                        psY.tile([P, T], F32, name=f"py{oc}", tag="py") for oc in range(2)
                    ]
                    pbs_pool = []
                    for h in range(NH):
                        pbp = psB.tile([32, T], F32, name="pb", tag="pb")
                        for qh in range(2):
                            nc.tensor.matmul(
                                pbp[:, qh * 512 : (qh + 1) * 512],
                                alphas[0:1, h * HD : (h + 1) * HD],
                                s1all[0:1, h * T + qh * 512 : h * T + (qh + 1) * 512],
                                start=True,
                                stop=True,
                            )
                        pbs = sp.tile([32, T], BF16, name="pbs", tag="pbs", bufs=2)
                        nc.scalar.activation(pbs[:], pbp[:], AF.Copy)
                        nc.vector.tensor_mul(
                            zr_sb[0:32, h * T : (h + 1) * T],
                            zr_sb[0:32, h * T : (h + 1) * T],
                            pbs[:],
                        )
                        if h == 3 or h == 7:
                            hf = h // 4
                            for oc in range(2):
                                for j in range(8):
                                    nc.tensor.matmul(
                                        py[oc][:, hf * 512 : (hf + 1) * 512],
                                        wptb_sb[:, j, oc * P : (oc + 1) * P],
                                        zrr[:, 4 * hf : 4 * hf + 4, :, j],
                                        start=(j == 0),
                                        stop=(j == 7),
                                        skip_group_check=True,
                                    )
                    # bias fixup rows -> [128, NH] broadcast via PE
                    pbb = psB.tile([P, NH], F32, name="pbb", tag="pb")
                    nc.tensor.matmul(
                        pbb[:], onesr128[:], biasrow[:], start=True, stop=True
                    )
                    for oc in range(2):
                        nc.vector.tensor_mul(u_sb[oc][:], u_sb[oc][:], pbb[:])
                        nc.vector.tensor_scalar_add(
                            u_sb[oc][:], u_sb[oc][:], bp_sb[oc][:, 0:1]
                        )
                    for hf in range(2):
                        for oc in range(2):
                            yv = y_sb[oc][:].rearrange("p (h m) -> p h m", h=NH)
                            pyv = py[oc][:].rearrange("p (h m) -> p h m", h=NH)
                            bias_b = u_sb[oc][:, 4 * hf : 4 * hf + 4, None].broadcast_to(
                                (P, 4, P)
                            )
                            nc.vector.tensor_add(
                                yv[:, 4 * hf : 4 * hf + 4, :],
                                pyv[:, 4 * hf : 4 * hf + 4, :],
                                bias_b,
                            )
                            eng = nc.sync if oc == 0 else nc.gpsimd
                            eng.dma_start(
                                yt_d[
                                    oc * P : (oc + 1) * P,
                                    hf * 512 : (hf + 1) * 512,
                                ],
                                y_sb[oc][:, hf * 512 : (hf + 1) * 512],
                            )

            for _rep in range(reps):
                _one_rep()

    _split_excess_waits(nc)
    return nc


def _host_inputs(x, Wq, Wk, Wv, w_head, gamma, beta, Wp, bp):
    import ml_dtypes

    f = np.float32
    common = {
        "wqt": np.ascontiguousarray(np.asarray(Wq, f).T),
        "wkt": np.ascontiguousarray(np.asarray(Wk, f).T),
        "wvt": np.ascontiguousarray(np.asarray(Wv, f).T),
        "wst": np.ascontiguousarray(np.repeat(np.asarray(w_head, f), HD, axis=1).T),
        "wptb": np.ascontiguousarray(
            np.asarray(Wp, f).T.reshape(8, HD, C).transpose(1, 0, 2)
        ).astype(ml_dtypes.bfloat16),
        "wpct": np.ascontiguousarray(
            np.asarray(Wp, f).T.reshape(8, HD, C).sum(0)
        ),
        "bp": np.ascontiguousarray(np.asarray(bp, f).reshape(C, 1)),
        "gmr": np.ascontiguousarray(np.asarray(gamma, f).reshape(1, NH)),
        "btr": np.ascontiguousarray(np.asarray(beta, f).reshape(1, NH)),
    }
    xs = np.asarray(x, f).reshape(B, C, T)
    return [
        {"xf": np.ascontiguousarray(xs[b]), **common} for b in range(B)
    ]


_NC_CACHE = {}


def _get_nc(reps=1):
    if reps not in _NC_CACHE:
        _NC_CACHE[reps] = build_bass(reps=reps)
    return _NC_CACHE[reps]


def run(inputs, trace=False):
    nc = _get_nc()
    in_maps = _host_inputs(**inputs)
    res = run_bass_kernel_spmd(
        nc, in_maps, core_ids=list(range(N_CORES)), trace=trace
    )
    y = np.stack([res.results[b]["yt"] for b in range(B)], axis=0)
    return y.reshape(B, C, 32, 32).astype(np.float32), res


def _build_sharded(reps=1):
    """Replicate bass2jax.run_bass_via_pjrt but return a reusable callable
    (no donation) so device execution can be timed over many iterations."""
    import jax
    from jax.sharding import Mesh, PartitionSpec
    from jax.experimental.shard_map import shard_map
    from concourse import bass2jax

    nc = _get_nc(reps)
    bass2jax.install_neuronx_cc_hook()
    part_name = nc.partition_id_tensor.name if nc.partition_id_tensor else None
    in_names, out_names, out_avals = [], [], []
    for alloc in nc.m.functions[0].allocations:
        if not isinstance(alloc, mybir.MemoryLocationSet):
            continue
        name = alloc.memorylocations[0].name
        if alloc.kind == "ExternalInput":
            if name == part_name:
                continue
            in_names.append(name)
        elif alloc.kind == "ExternalOutput":
            out_names.append(name)
            out_avals.append(
                jax.core.ShapedArray(
                    tuple(alloc.tensor_shape), mybir.dt.np(alloc.dtype)
                )
            )
    n_params = len(in_names)
    all_in = in_names + out_names
    if part_name is not None:
        all_in = all_in + [part_name]

    def _body(*args):
        operands = list(args)
        if part_name is not None:
            operands.append(bass2jax.partition_id_tensor())
        outs = bass2jax._bass_exec_p.bind(
            *operands,
            out_avals=tuple(out_avals),
            in_names=tuple(all_in),
            out_names=tuple(out_names),
            lowering_input_output_aliases=(),
            sim_require_finite=True,
            sim_require_nnan=True,
            nc=nc,
        )
        return tuple(outs)

    devices = jax.devices()[:N_CORES]
    mesh = Mesh(np.asarray(devices), ("core",))
    nouts = len(out_names)
    sharded = jax.jit(
        shard_map(
            _body,
            mesh=mesh,
            in_specs=(PartitionSpec("core"),) * (n_params + nouts),
            out_specs=(PartitionSpec("core"),) * nouts,
            check_rep=False,
        ),
        keep_unused=True,
    )
    return sharded, mesh, in_names, out_names, out_avals


def timed_run(inputs, iters=20, reps=1):
    import time
    import jax
    from jax.sharding import NamedSharding, PartitionSpec

    sharded, mesh, in_names, out_names, out_avals = _build_sharded(reps)
    in_maps = _host_inputs(**inputs)
    sh = NamedSharding(mesh, PartitionSpec("core"))
    dev_in = [
        jax.device_put(
            np.concatenate([in_maps[c][n] for c in range(N_CORES)], axis=0), sh
        )
        for n in in_names
    ]
    dev_zero = [
        jax.device_put(
            np.zeros((N_CORES * a.shape[0], *a.shape[1:]), a.dtype), sh
        )
        for a in out_avals
    ]
    out = sharded(*dev_in, *dev_zero)
    jax.block_until_ready(out)
    # blocking per-call (includes full dispatch round trip)
    times = []
    for _ in range(max(3, iters // 4)):
        t0 = time.perf_counter()
        out = sharded(*dev_in, *dev_zero)
        jax.block_until_ready(out)
        times.append(time.perf_counter() - t0)
    # pipelined: submit all, block once -> amortizes host/axon dispatch
    t0 = time.perf_counter()
    outs = [sharded(*dev_in, *dev_zero) for _ in range(iters)]
    jax.block_until_ready(outs)
    pipelined = (time.perf_counter() - t0) / iters
    times.append(pipelined)
    print(f"pipelined per-call: {pipelined * 1e9:.0f} ns")
    y = np.asarray(outs[-1][out_names.index("yt")]).reshape(N_CORES, C, T)
    return y.reshape(B, C, 32, 32).astype(np.float32), times


def kernel(**inputs):
    y, _ = run(inputs, trace=False)
    return y


def numpy_check():
    """CoreSim single-core check against a numpy reference (core 0 data)."""
    from concourse.bass_interp import CoreSim

    rng = np.random.default_rng(0)
    x = rng.standard_normal((B, C, 32, 32), np.float32)
    Wq = (rng.standard_normal((C, C)) * 0.05).astype(np.float32)
    Wk = (rng.standard_normal((C, C)) * 0.05).astype(np.float32)
    Wv = (rng.standard_normal((C, C)) * 0.05).astype(np.float32)
    w_head = (rng.standard_normal((NH, NH)) * 0.3).astype(np.float32)
    gamma = rng.uniform(0.5, 1.5, NH).astype(np.float32)
    beta = (rng.standard_normal(NH) * 0.1).astype(np.float32)
    Wp = (rng.standard_normal((C, C)) * 0.05).astype(np.float32)
    bp = (rng.standard_normal(C) * 0.05).astype(np.float32)
    inputs = dict(
        x=x, Wq=Wq, Wk=Wk, Wv=Wv, w_head=w_head, gamma=gamma, beta=beta,
        Wp=Wp, bp=bp,
    )

    def ref_np(x, Wq, Wk, Wv, w_head, gamma, beta, Wp, bp):
        Bn, Cn, H, W = x.shape
        Tn = H * W
        hd = Cn // NH
        sc = float(hd) ** -0.5
        xf = x.reshape(Bn, Cn, Tn).astype(np.float64)
        q = np.einsum("oc,bct->bot", Wq, xf).reshape(Bn, NH, hd, Tn)
        k = np.einsum("oc,bct->bot", Wk, xf).reshape(Bn, NH, hd, Tn)
        v = np.einsum("oc,bct->bot", Wv, xf).reshape(Bn, NH, hd, Tn)
        s = np.einsum("bhdq,bhdt->bhqt", q, k) * sc
        s = np.einsum("hg,bgqt->bhqt", w_head.astype(np.float64), s)
        s = s - s.max(axis=-1, keepdims=True)
        e = np.exp(s)
        a = e / e.sum(-1, keepdims=True)
        mean = a.mean(axis=(2, 3), keepdims=True)
        var = a.var(axis=(2, 3), keepdims=True)
        g = gamma.astype(np.float64)[None, :, None, None]
        bt = beta.astype(np.float64)[None, :, None, None]
        a = (a - mean) / np.sqrt(var + EPS) * g + bt
        out = np.einsum("bhqt,bhdt->bhqd", a, v)
        y = out.reshape(Bn, Tn, Cn)
        y = np.einsum("btc,oc->bto", y, Wp.astype(np.float64)) + bp
        return y.transpose(0, 2, 1).reshape(Bn, Cn, H, W)

    expected = ref_np(**inputs)[0]  # core 0

    nc = _get_nc()
    in_maps = _host_inputs(**inputs)
    sim = CoreSim(nc, trace=False)
    for name, arr in in_maps[0].items():
        sim.tensor(name)[:] = arr
    sim.simulate(check_with_hw=False)
    got = np.array(sim.tensor("yt")).reshape(C, 32, 32)
    err = np.abs(got - expected) / (np.abs(expected) + 1e-3)
    scale = np.abs(got - expected).max() / np.abs(expected).max()
    print("max rel err (sim vs numpy f64):", err.max())
    print("mean rel err:", err.mean())
    print("scale-relative absmax:", scale)
    return err.max()


if __name__ == "__main__":
    numpy_check()
